# revision 15
# baseline (speedup 1.0000x reference)
"""Chamfer loss kernel for Trainium2 (Bass/Tile), 8-core SPMD.

Problem: B=16 batches of pred[4096,64] / target[4096,64] fp32.
  chamfer_b = mean_n min_m ||x_n - y_m||^2 + mean_m min_n ||x_n - y_m||^2
  output    = mean_b chamfer_b  (scalar fp32)

Sharding: data-parallel over batch, 2 batches per core.

Per-core algorithm (per batch):
  The negated squared-distance matrix  -D[n,m] = 2 x.y - ||x||^2 - ||y||^2
  is materialized tile-by-tile in PSUM via three accumulated matmul passes
  (bf16 hi/lo split of the fp32 inputs for near-fp32 accuracy at bf16 PE
  throughput):
    pass A (K=64):  xh^T . (2*yh)
    pass B (K=128): [xh;xl]^T . [2*yl;2*yh]   (cross terms)
    pass C (K=4):   [sqxh;sqxl;1;1]^T . [-1;-1;-sqyh;-sqyl]  (norm terms)
  Each PSUM unit is [128, 2048] (one n-tile x half the m range, 4 banks).
  DVE then does, per unit:
    row direction: reduce_max over the free axis  -> -min_m D
    col direction: running elementwise max into colrun[128, 4096]
  Endgame: colrun partition-axis max via PE transposes + reduce, sums of
  both directions -> per-lane partial sums [128, bpc] written to DRAM.
Host: sums partials over lanes/batches/cores, negates, divides.
"""

import os
from contextlib import ExitStack

import numpy as np

import concourse.bass as bass
import concourse.mybir as mybir
from concourse import bacc
from concourse.tile import TileContext
from concourse.bass_utils import run_bass_kernel_spmd
from concourse.masks import make_identity

F32 = mybir.dt.float32
BF16 = mybir.dt.bfloat16
AX = mybir.AxisListType
OP = mybir.AluOpType
P = 128
BANK_F32 = 512          # fp32 elems per PSUM bank
UNIT_W = 2048           # unit width in m (4 banks)

B_FULL, N_FULL, M_FULL, D_FULL = 16, 4096, 4096, 64
NCORES = 8
BPC = B_FULL // NCORES  # batches per core


def emit_chamfer(tc, pred, targ, out, bpc, n, m, d):
    """Emit the chamfer program. pred/targ: [bpc, n|m, d] f32 DRAM APs.
    out: [128, bpc] f32 DRAM AP receiving per-lane sums of
    (rowmax + colmax) of the negated distance matrix."""
    nc = tc.nc
    nt = n // P                 # n-tiles
    unit_w = min(UNIT_W, m)
    nu = m // unit_w            # units per n-tile
    nb = unit_w // BANK_F32     # banks (matmuls) per unit per pass
    mt = m // P                 # m blocks of 128 (endgame transposes)
    tpc = unit_w // P           # transposes per endgame chunk

    ctx = ExitStack()
    const = ctx.enter_context(tc.tile_pool(name="const", bufs=1))
    bpool = ctx.enter_context(tc.tile_pool(name="batch", bufs=2))
    ppool = ctx.enter_context(tc.tile_pool(name="psum", bufs=2, space="PSUM"))
    dpool = ctx.enter_context(tc.tile_pool(name="dram", bufs=2, space="DRAM"))
    opool = ctx.enter_context(tc.tile_pool(name="outp", bufs=1))

    ident = const.tile([P, P], F32, tag="ident")
    make_identity(nc, ident[:])

    import ml_dtypes
    ones_np = np.ones((4, max(n, m)), dtype=ml_dtypes.bfloat16)
    ones_np[2:4] = -1.0
    const_ones = nc.inline_tensor(ones_np, name="const_ones").ap()

    totals = opool.tile([P, bpc], F32, tag="totals")

    for b in range(bpc):
        # ---- load inputs (n-major: lane p, tile t -> point t*128+p) ----
        x = bpool.tile([P, nt, d], F32, tag="x")
        nc.sync.dma_start(x[:], pred[b].rearrange("(t p) d -> p t d", p=P))
        ytiles = m // P
        y = bpool.tile([P, ytiles, d], F32, tag="y")
        nc.sync.dma_start(y[:], targ[b].rearrange("(t p) d -> p t d", p=P))

        # ---- squared norms (fp32) ----
        xsq = bpool.tile([P, nt, d], F32, tag="sqtmp")
        nc.scalar.square(xsq[:], x[:])
        sqx = bpool.tile([P, nt], F32, tag="sqx")
        nc.vector.tensor_reduce(sqx[:], xsq[:], axis=AX.X, op=OP.add)
        ysq = bpool.tile([P, ytiles, d], F32, tag="sqtmp")
        nc.scalar.square(ysq[:], y[:])
        sqy = bpool.tile([P, ytiles], F32, tag="sqy")
        nc.vector.tensor_reduce(sqy[:], ysq[:], axis=AX.X, op=OP.add)

        # ---- bf16 hi/lo splits (x side carries the 2x scale in place) ----
        xh = bpool.tile([P, nt, d], BF16, tag="xh")
        nc.vector.tensor_copy(out=xh[:], in_=x[:])
        xl = bpool.tile([P, nt, d], BF16, tag="xl")
        nc.vector.tensor_tensor(xl[:], x[:], xh[:], OP.subtract)
        nc.vector.tensor_scalar_mul(xh[:], xh[:], 2.0)
        nc.vector.tensor_scalar_mul(xl[:], xl[:], 2.0)
        yh = bpool.tile([P, ytiles, d], BF16, tag="yh")
        nc.vector.tensor_copy(out=yh[:], in_=y[:])
        yl = bpool.tile([P, ytiles, d], BF16, tag="yl")
        nc.vector.tensor_tensor(yl[:], y[:], yh[:], OP.subtract)

        # ---- transpose to d-major via DRAM scratch + DMA-transpose ----
        sx = dpool.tile([n, 2 * d], BF16, tag="sx")
        nc.sync.dma_start(sx[:, 0:d].rearrange("(t p) d -> p t d", p=P), xh[:])
        nc.sync.dma_start(sx[:, d:2 * d].rearrange("(t p) d -> p t d", p=P), xl[:])
        sy = dpool.tile([m, 2 * d], BF16, tag="sy")
        nc.sync.dma_start(sy[:, 0:d].rearrange("(t p) d -> p t d", p=P), yl[:])
        nc.sync.dma_start(sy[:, d:2 * d].rearrange("(t p) d -> p t d", p=P), yh[:])

        uB = bpool.tile([2 * d, n], BF16, tag="uB")   # [2xh^T; 2xl^T]
        nc.sync.dma_start(uB[:], sx[:], transpose=True)
        vB = bpool.tile([2 * d, m], BF16, tag="vB")   # [yl^T; yh^T]
        nc.sync.dma_start(vB[:], sy[:], transpose=True)
        # pass A rhs must share base partition 0 with uB[0:d] -> own tensor
        vA = bpool.tile([d, m], BF16, tag="vA")       # [yh^T]
        nc.sync.dma_start(vA[:], vB[d:2 * d, :])

        # ---- norm-term lift rows (pass C operands) ----
        sqxh = bpool.tile([P, nt], BF16, tag="sqxh")
        nc.vector.tensor_copy(out=sqxh[:], in_=sqx[:])
        sqxl = bpool.tile([P, nt], BF16, tag="sqxl")
        nc.vector.tensor_tensor(sqxl[:], sqx[:], sqxh[:], OP.subtract)
        sqyh = bpool.tile([P, ytiles], BF16, tag="sqyh")
        nc.vector.tensor_copy(out=sqyh[:], in_=sqy[:])
        sqyl = bpool.tile([P, ytiles], BF16, tag="sqyl")
        nc.vector.tensor_tensor(sqyl[:], sqy[:], sqyh[:], OP.subtract)
        sqyhn = bpool.tile([P, ytiles], BF16, tag="sqyhn")
        nc.vector.tensor_scalar_mul(sqyhn[:], sqyh[:], -1.0)
        sqyln = bpool.tile([P, ytiles], BF16, tag="sqyln")
        nc.vector.tensor_scalar_mul(sqyln[:], sqyl[:], -1.0)

        # Assemble the 4-row lift blocks in DRAM (partition-aligned SBUF
        # writes only), then load each with a single partition-0 DMA.
        # Constant +-1 rows come from an inline (NEFF-embedded) tensor.
        srx = dpool.tile([4, n], BF16, tag="srx")
        sry = dpool.tile([4, m], BF16, tag="sry")
        with nc.allow_non_contiguous_dma(reason="small sq-row scatter"):
            nc.sync.dma_start(srx[0].rearrange("(t p) -> p t", p=P), sqxh[:])
            nc.sync.dma_start(srx[1].rearrange("(t p) -> p t", p=P), sqxl[:])
            nc.sync.dma_start(sry[2].rearrange("(t p) -> p t", p=P), sqyhn[:])
            nc.sync.dma_start(sry[3].rearrange("(t p) -> p t", p=P), sqyln[:])
        nc.sync.dma_start(srx[2:4, :], const_ones[0:2, :n])
        nc.sync.dma_start(sry[0:2, :], const_ones[2:4, :m])
        uC = bpool.tile([4, n], BF16, tag="uC")
        vC = bpool.tile([4, m], BF16, tag="vC")
        nc.sync.dma_start(uC[:], srx[:])
        nc.sync.dma_start(vC[:], sry[:])

        # ---- main loop: materialize -D in PSUM, reduce ----
        colrun = bpool.tile([P, m], F32, tag="colrun")
        rm = bpool.tile([P, nt * nu], F32, tag="rm")

        for i in range(nt):
            nsl = slice(i * P, (i + 1) * P)
            for h in range(nu):
                pt = ppool.tile([P, unit_w], F32, tag="pt")
                base = h * unit_w
                for j in range(nb):
                    bs = slice(j * BANK_F32, (j + 1) * BANK_F32)
                    ms = slice(base + j * BANK_F32, base + (j + 1) * BANK_F32)
                    nc.tensor.matmul(
                        pt[:, bs], uB[0:d, nsl], vA[:, ms],
                        start=True, stop=False)
                for j in range(nb):
                    bs = slice(j * BANK_F32, (j + 1) * BANK_F32)
                    ms = slice(base + j * BANK_F32, base + (j + 1) * BANK_F32)
                    nc.tensor.matmul(
                        pt[:, bs], uB[:, nsl], vB[:, ms],
                        start=False, stop=False)
                for j in range(nb):
                    bs = slice(j * BANK_F32, (j + 1) * BANK_F32)
                    ms = slice(base + j * BANK_F32, base + (j + 1) * BANK_F32)
                    nc.tensor.matmul(
                        pt[:, bs], uC[:, nsl], vC[:, ms],
                        start=False, stop=True)
                k = i * nu + h
                nc.vector.tensor_reduce(
                    rm[:, k:k + 1], pt[:], axis=AX.X, op=OP.max)
                csl = slice(base, base + unit_w)
                if i == 0:
                    nc.vector.tensor_copy(out=colrun[:, csl], in_=pt[:])
                else:
                    nc.vector.tensor_tensor(
                        colrun[:, csl], pt[:], colrun[:, csl], OP.max)

        # ---- endgame ----
        # row direction: max over the nu unit maxes, then sum over n
        if nu > 1:
            rm3 = rm[:].rearrange("p (t u) -> p t u", u=nu)
            rmx = bpool.tile([P, nt], F32, tag="rmx")
            nc.vector.tensor_tensor(rmx[:], rm3[:, :, 0], rm3[:, :, 1], OP.max)
        else:
            rmx = rm
        rsum = bpool.tile([P, 1], F32, tag="rsum")
        nc.vector.tensor_reduce(rsum[:], rmx[:], axis=AX.X, op=OP.add)

        # col direction: partition-axis max via PE transpose, then sum over m
        cm = bpool.tile([P, mt], F32, tag="cm")
        for c in range(nu):
            ptt = ppool.tile([P, unit_w], F32, tag="pt")
            for t in range(tpc):
                msl = slice(c * unit_w + t * P, c * unit_w + (t + 1) * P)
                nc.tensor.matmul(
                    ptt[:, t * P:(t + 1) * P], colrun[:, msl], ident[:],
                    is_transpose=True,
                    start=(t % 4 == 0), stop=(t % 4 == 3))
            nc.vector.tensor_reduce(
                cm[:, c * tpc:(c + 1) * tpc],
                ptt[:].rearrange("p (t q) -> p t q", q=P),
                axis=AX.X, op=OP.max)
        csum = bpool.tile([P, 1], F32, tag="csum")
        nc.vector.tensor_reduce(csum[:], cm[:], axis=AX.X, op=OP.add)

        nc.vector.tensor_tensor(totals[:, b:b + 1], rsum[:], csum[:], OP.add)

    nc.sync.dma_start(out[:], totals[:])
    ctx.close()


def build_program(bpc=BPC, n=N_FULL, m=M_FULL, d=D_FULL, debug=False):
    nc = bacc.Bacc(
        "TRN2", target_bir_lowering=False, debug=debug, enable_asserts=False)
    pred = nc.dram_tensor("pred_in", (bpc, n, d), F32, kind="ExternalInput").ap()
    targ = nc.dram_tensor("targ_in", (bpc, m, d), F32, kind="ExternalInput").ap()
    out = nc.dram_tensor("partials", (P, bpc), F32, kind="ExternalOutput").ap()
    with TileContext(nc) as tc:
        emit_chamfer(tc, pred, targ, out, bpc, n, m, d)
    nc.compile()
    return nc


_NC_CACHE = {}


def _get_program():
    key = (BPC, N_FULL, M_FULL, D_FULL)
    if key not in _NC_CACHE:
        _NC_CACHE[key] = build_program(*key)
    return _NC_CACHE[key]


def kernel(pred_set, target_set):
    pred = np.ascontiguousarray(np.asarray(pred_set, dtype=np.float32))
    targ = np.ascontiguousarray(np.asarray(target_set, dtype=np.float32))
    assert pred.shape == (B_FULL, N_FULL, D_FULL), pred.shape
    assert targ.shape == (B_FULL, M_FULL, D_FULL), targ.shape

    nc = _get_program()
    in_maps = [
        {
            "pred_in": pred[c * BPC:(c + 1) * BPC],
            "targ_in": targ[c * BPC:(c + 1) * BPC],
        }
        for c in range(NCORES)
    ]
    trace = bool(int(os.environ.get("CHAMFER_TRACE", "0")))
    res = run_bass_kernel_spmd(
        nc, in_maps, core_ids=list(range(NCORES)), trace=trace)
    kernel.last_results = res
    total = 0.0
    for r in res.results:
        total += float(r["partials"].astype(np.float64).sum())
    val = -total / (float(N_FULL) * B_FULL)
    return np.float32(val)


# revision 37
# speedup vs baseline: 4262.2962x; 4262.2962x over previous
"""Chamfer loss kernel for Trainium2 (Bass/Tile), 8-core SPMD.

Problem: B=16 batches of pred[4096,64] / target[4096,64] fp32.
  chamfer_b = mean_n min_m ||x_n - y_m||^2 + mean_m min_n ||x_n - y_m||^2
  output    = mean_b chamfer_b  (scalar fp32)

Sharding: data-parallel over batch, 2 batches per core.

Per-core algorithm (per batch):
  The negated squared-distance matrix  -D[n,m] = 2 x.y - ||x||^2 - ||y||^2
  is materialized tile-by-tile in PSUM via three accumulated matmul passes
  (bf16 hi/lo split of the fp32 inputs for near-fp32 accuracy at bf16 PE
  throughput):
    pass A (K=64):  xh^T . (2*yh)
    pass B (K=128): [xh;xl]^T . [2*yl;2*yh]   (cross terms)
    pass C (K=4):   [sqxh;sqxl;1;1]^T . [-1;-1;-sqyh;-sqyl]  (norm terms)
  Each PSUM unit is [128, 2048] (one n-tile x half the m range, 4 banks).
  DVE then does, per unit:
    row direction: reduce_max over the free axis  -> -min_m D
    col direction: running elementwise max into colrun[128, 4096]
  Endgame: colrun partition-axis max via PE transposes + reduce, sums of
  both directions -> per-lane partial sums [128, bpc] written to DRAM.
Host: sums partials over lanes/batches/cores, negates, divides.
"""

import os
from contextlib import ExitStack

import numpy as np

import concourse.bass as bass
import concourse.mybir as mybir
from concourse import bacc
from concourse.tile import TileContext
from concourse.bass_utils import run_bass_kernel_spmd
from concourse.masks import make_identity

F32 = mybir.dt.float32
BF16 = mybir.dt.bfloat16
AX = mybir.AxisListType
OP = mybir.AluOpType
P = 128
BANK_F32 = 512          # fp32 elems per PSUM bank
UNIT_W = 2048           # unit width in m (4 banks)

B_FULL, N_FULL, M_FULL, D_FULL = 16, 4096, 4096, 64
NCORES = 8
BPC = B_FULL // NCORES  # batches per core

# Colmax offload: most units' running-max goes to GPSIMD (fed by an ACT
# PSUM->SBUF stage copy); every DVE_COLMAX_EVERY-th n-tile stays on DVE
# (own accumulator chain, merged at the end). 0 disables GPSIMD offload.
ROWMAX_TS = bool(int(os.environ.get("CHAMFER_TS", "0")))
PREPROC_PRIO_OFFSET = int(os.environ.get("CHAMFER_PRIO", "700"))


def emit_chamfer(tc, pred, targ, out, bpc, n, m, d):
    """Emit the chamfer program. pred/targ: [bpc, n|m, d] f32 DRAM APs.
    out: [128, bpc] f32 DRAM AP receiving per-lane sums of
    (rowmax + colmax) of the negated distance matrix."""
    nc = tc.nc
    nt = n // P                 # n-tiles
    unit_w = min(UNIT_W, m)
    nu = m // unit_w            # units per n-tile
    nb = unit_w // BANK_F32     # banks (matmuls) per unit per pass
    mt = m // P                 # m blocks of 128 (endgame transposes)
    tpc = unit_w // P           # transposes per endgame chunk

    ctx = ExitStack()
    const = ctx.enter_context(tc.tile_pool(name="const", bufs=1))
    bpool = ctx.enter_context(tc.tile_pool(name="batch", bufs=2))
    cpool = ctx.enter_context(tc.tile_pool(name="colr", bufs=1))
    ppool = ctx.enter_context(tc.tile_pool(name="psum", bufs=2, space="PSUM"))
    dpool = ctx.enter_context(tc.tile_pool(name="dram", bufs=2, space="DRAM"))
    opool = ctx.enter_context(tc.tile_pool(name="outp", bufs=1))

    ident = const.tile([P, P], F32, tag="ident")
    make_identity(nc, ident[:])

    import ml_dtypes
    # Constant pads for the transposed lift-operand scratch buffers:
    # sxa cols 66:128 = [1, 1, 0...]; sya cols 64:128 = [-1, -1, *, *, 0...]
    px_np = np.zeros((n, 62), dtype=ml_dtypes.bfloat16)
    px_np[:, 0:2] = 1.0
    const_px = nc.inline_tensor(px_np, name="const_px").ap()
    py_np = np.zeros((m, 64), dtype=ml_dtypes.bfloat16)
    py_np[:, 0:2] = -1.0
    const_py = nc.inline_tensor(py_np, name="const_py").ap()

    totals = opool.tile([P, bpc], F32, tag="totals")

    for b in range(bpc):
        # Let batch b>0's preprocessing interleave with the previous batch's
        # unit phase: pull its scheduler priority back into that range.
        if b > 0:
            _saved_prio = tc.cur_priority
            tc.cur_priority = max(0, _saved_prio - PREPROC_PRIO_OFFSET)
        # ---- load inputs (n-major: lane p, tile t -> point t*128+p) ----
        x = bpool.tile([P, nt, d], F32, tag="x")
        nc.sync.dma_start(x[:], pred[b].rearrange("(t p) d -> p t d", p=P))
        ytiles = m // P
        ydma = nc.scalar if b == 0 else nc.sync
        y = bpool.tile([P, ytiles, d], F32, tag="y")
        ydma.dma_start(y[:], targ[b].rearrange("(t p) d -> p t d", p=P))

        # ---- squared norms (fp32), chunked so the temp has its own small
        # tag (sharing the "stage" tag would serialize against the previous
        # batch's unit-phase stage tiles) ----
        CH = 8
        sqx = bpool.tile([P, nt], F32, tag="sqx")
        for c0 in range(0, nt, CH):
            cw = min(CH, nt - c0)
            tmp = bpool.tile([P, CH, d], F32, tag="sqtmp")
            nc.scalar.square(tmp[:, :cw], x[:, c0:c0 + cw])
            nc.vector.tensor_reduce(
                sqx[:, c0:c0 + cw], tmp[:, :cw], axis=AX.X, op=OP.add)
        sqy = bpool.tile([P, ytiles], F32, tag="sqy")
        for c0 in range(0, ytiles, CH):
            cw = min(CH, ytiles - c0)
            tmp = bpool.tile([P, CH, d], F32, tag="sqtmp")
            nc.scalar.square(tmp[:, :cw], y[:, c0:c0 + cw])
            nc.vector.tensor_reduce(
                sqy[:, c0:c0 + cw], tmp[:, :cw], axis=AX.X, op=OP.add)

        # ---- bf16 hi/lo splits (x side carries the 2x scale in place) ----
        xh = bpool.tile([P, nt, d], BF16, tag="xh")
        nc.scalar.copy(xh[:], x[:])              # cast on ACT
        xl = bpool.tile([P, nt, d], BF16, tag="xl")
        nc.vector.tensor_tensor(xl[:], x[:], xh[:], OP.subtract)
        nc.vector.tensor_scalar_mul(xh[:], xh[:], 2.0)
        nc.vector.tensor_scalar_mul(xl[:], xl[:], 2.0)
        yh = bpool.tile([P, ytiles, d], BF16, tag="yh")
        nc.scalar.copy(yh[:], y[:])              # cast on ACT
        yl = bpool.tile([P, ytiles, d], BF16, tag="yl")
        nc.vector.tensor_tensor(yl[:], y[:], yh[:], OP.subtract)

        # ---- transpose to d-major via DRAM scratch + DMA-transpose ----
        sx = dpool.tile([n, 2 * d], BF16, tag="sx")
        nc.sync.dma_start(sx[:, 0:d].rearrange("(t p) d -> p t d", p=P), xh[:])
        nc.sync.dma_start(sx[:, d:2 * d].rearrange("(t p) d -> p t d", p=P), xl[:])
        sy = dpool.tile([m, 2 * d], BF16, tag="sy")
        ydma.dma_start(sy[:, 0:d].rearrange("(t p) d -> p t d", p=P), yl[:])
        ydma.dma_start(sy[:, d:2 * d].rearrange("(t p) d -> p t d", p=P), yh[:])

        uB = bpool.tile([2 * d, n], BF16, tag="uB")   # [2xh^T; 2xl^T]
        nc.sync.dma_start(uB[:], sx[:], transpose=True)
        vB = bpool.tile([2 * d, m], BF16, tag="vB")   # [yl^T; yh^T]
        ydma.dma_start(vB[:], sy[:], transpose=True)

        # ---- norm-term lift rows ----
        sqxh = bpool.tile([P, nt], BF16, tag="sqxh")
        nc.vector.tensor_copy(out=sqxh[:], in_=sqx[:])
        sqxl = bpool.tile([P, nt], BF16, tag="sqxl")
        nc.vector.tensor_tensor(sqxl[:], sqx[:], sqxh[:], OP.subtract)
        sqyh = bpool.tile([P, ytiles], BF16, tag="sqyh")
        nc.vector.tensor_copy(out=sqyh[:], in_=sqy[:])
        sqyl = bpool.tile([P, ytiles], BF16, tag="sqyl")
        nc.vector.tensor_tensor(sqyl[:], sqy[:], sqyh[:], OP.subtract)
        sqyhn = bpool.tile([P, ytiles], BF16, tag="sqyhn")
        nc.vector.tensor_scalar_mul(sqyhn[:], sqyh[:], -1.0)
        sqyln = bpool.tile([P, ytiles], BF16, tag="sqyln")
        nc.vector.tensor_scalar_mul(sqyln[:], sqyl[:], -1.0)

        # Pass-A operands come straight from a second DMA-transpose whose
        # DRAM source carries the lift rows as extra columns:
        #   sxa = [2xh | sqxh | sqxl | 1 | 1 | 0...]   -> uA rows 0..67
        #   sya = [yh  | -1   | -1   | -sqyh | -sqyl | 0...] -> vA rows 0..67
        sxa = dpool.tile([n, P], BF16, tag="sxa")
        nc.sync.dma_start(sxa[:, 0:d].rearrange("(t p) d -> p t d", p=P), xh[:])
        nc.sync.dma_start(sxa[:, d + 2:P], const_px[:, :])
        sya = dpool.tile([m, P], BF16, tag="sya")
        ydma.dma_start(sya[:, 0:d].rearrange("(t p) d -> p t d", p=P), yh[:])
        ydma.dma_start(sya[:, d:P], const_py[:, :])
        with nc.allow_non_contiguous_dma(reason="small sq-row scatter"):
            nc.sync.dma_start(
                sxa[:, d].rearrange("(t p) -> p t", p=P), sqxh[:])
            nc.sync.dma_start(
                sxa[:, d + 1].rearrange("(t p) -> p t", p=P), sqxl[:])
            ydma.dma_start(
                sya[:, d + 2].rearrange("(t p) -> p t", p=P), sqyhn[:])
            ydma.dma_start(
                sya[:, d + 3].rearrange("(t p) -> p t", p=P), sqyln[:])
        uA = bpool.tile([P, n], BF16, tag="uA")
        nc.sync.dma_start(uA[:], sxa[:], transpose=True)
        vA = bpool.tile([P, m], BF16, tag="vA")
        ydma.dma_start(vA[:], sya[:], transpose=True)

        if b > 0:
            tc.cur_priority = _saved_prio

        # ---- main loop: materialize -D in PSUM, reduce ----
        # Per unit: ACT stages PSUM->SBUF; DVE then does the rowmax as a
        # single-source tensor_scalar+accum on the SBUF stage (2x_2p mode,
        # 2 elem/cyc) and the colmax running max as an SBUF tensor_tensor.
        use_gps = False
        colrun = cpool.tile([P, m], F32, tag="colrunG", name="colrunG")
        rm = bpool.tile([P, nt * nu], F32, tag="rm")
        seen = [False] * nu

        for i in range(nt):
            nsl = slice(i * P, (i + 1) * P)
            for h in range(nu):
                pt = ppool.tile([P, unit_w], F32, tag="pt")
                base = h * unit_w
                for j in range(nb):
                    bs = slice(j * BANK_F32, (j + 1) * BANK_F32)
                    ms = slice(base + j * BANK_F32, base + (j + 1) * BANK_F32)
                    nc.tensor.matmul(
                        pt[:, bs], uB[:, nsl], vB[:, ms],
                        start=True, stop=False)
                for j in range(nb):
                    bs = slice(j * BANK_F32, (j + 1) * BANK_F32)
                    ms = slice(base + j * BANK_F32, base + (j + 1) * BANK_F32)
                    nc.tensor.matmul(
                        pt[:, bs], uA[0:d + 4, nsl], vA[0:d + 4, ms],
                        start=False, stop=True)
                k = i * nu + h
                csl = slice(base, base + unit_w)
                stage = bpool.tile([P, unit_w], F32, tag="stage")
                nc.scalar.copy(stage[:], pt[:])
                if ROWMAX_TS:
                    stage2 = bpool.tile([P, unit_w], F32, tag="stage2")
                    nc.vector.tensor_scalar(
                        out=stage2[:], in0=stage[:], scalar1=-3.0e38,
                        scalar2=None, op0=OP.max, op1=OP.max,
                        accum_out=rm[:, k:k + 1])
                else:
                    nc.vector.tensor_reduce(
                        rm[:, k:k + 1], pt[:], axis=AX.X, op=OP.max)
                if not seen[h]:
                    nc.vector.tensor_copy(out=colrun[:, csl], in_=stage[:])
                    seen[h] = True
                else:
                    nc.vector.tensor_tensor(
                        colrun[:, csl], stage[:], colrun[:, csl], OP.max)

        # ---- endgame ----
        # row direction: max over the nu unit maxes, then sum over n
        if nu > 1:
            rm3 = rm[:].rearrange("p (t u) -> p t u", u=nu)
            rmx = bpool.tile([P, nt], F32, tag="rmx")
            nc.vector.tensor_tensor(rmx[:], rm3[:, :, 0], rm3[:, :, 1], OP.max)
        else:
            rmx = rm
        rsum = bpool.tile([P, 1], F32, tag="rsum")
        nc.vector.tensor_reduce(rsum[:], rmx[:], axis=AX.X, op=OP.add)

        # col direction: merge the DVE-side accumulator chunkwise, then
        # partition-axis max via PE transpose + reduce (per chunk, so it
        # overlaps the tail of the unit phase)
        cm = bpool.tile([P, mt], F32, tag="cm")
        for c in range(nu):
            csl = slice(c * unit_w, (c + 1) * unit_w)
            if use_gps:
                nc.vector.tensor_tensor(
                    colrun[:, csl], colrunD[:, csl], colrun[:, csl], OP.max)
            ptt = ppool.tile([P, unit_w], F32, tag="pt")
            for t in range(tpc):
                msl = slice(c * unit_w + t * P, c * unit_w + (t + 1) * P)
                nc.tensor.matmul(
                    ptt[:, t * P:(t + 1) * P], colrun[:, msl], ident[:],
                    is_transpose=True,
                    start=(t % 4 == 0), stop=(t % 4 == 3))
            nc.vector.tensor_reduce(
                cm[:, c * tpc:(c + 1) * tpc],
                ptt[:].rearrange("p (t q) -> p t q", q=P),
                axis=AX.X, op=OP.max)
        csum = bpool.tile([P, 1], F32, tag="csum")
        nc.vector.tensor_reduce(csum[:], cm[:], axis=AX.X, op=OP.add)

        nc.vector.tensor_tensor(totals[:, b:b + 1], rsum[:], csum[:], OP.add)

    nc.sync.dma_start(out[:], totals[:])
    ctx.close()


def build_program(bpc=BPC, n=N_FULL, m=M_FULL, d=D_FULL, debug=False):
    nc = bacc.Bacc(
        "TRN2", target_bir_lowering=False, debug=debug, enable_asserts=False)
    pred = nc.dram_tensor("pred_in", (bpc, n, d), F32, kind="ExternalInput").ap()
    targ = nc.dram_tensor("targ_in", (bpc, m, d), F32, kind="ExternalInput").ap()
    out = nc.dram_tensor("partials", (P, bpc), F32, kind="ExternalOutput").ap()
    with TileContext(nc, pool_alloc_mode="queue") as tc:
        emit_chamfer(tc, pred, targ, out, bpc, n, m, d)
    nc.compile()
    return nc


_NC_CACHE = {}


def _get_program():
    key = (BPC, N_FULL, M_FULL, D_FULL)
    if key not in _NC_CACHE:
        _NC_CACHE[key] = build_program(*key)
    return _NC_CACHE[key]


def kernel(pred_set, target_set):
    pred = np.ascontiguousarray(np.asarray(pred_set, dtype=np.float32))
    targ = np.ascontiguousarray(np.asarray(target_set, dtype=np.float32))
    assert pred.shape == (B_FULL, N_FULL, D_FULL), pred.shape
    assert targ.shape == (B_FULL, M_FULL, D_FULL), targ.shape

    nc = _get_program()
    in_maps = [
        {
            "pred_in": pred[c * BPC:(c + 1) * BPC],
            "targ_in": targ[c * BPC:(c + 1) * BPC],
        }
        for c in range(NCORES)
    ]
    trace = bool(int(os.environ.get("CHAMFER_TRACE", "0")))
    res = run_bass_kernel_spmd(
        nc, in_maps, core_ids=list(range(NCORES)), trace=trace)
    kernel.last_results = res
    total = 0.0
    for r in res.results:
        total += float(r["partials"].astype(np.float64).sum())
    val = -total / (float(N_FULL) * B_FULL)
    return np.float32(val)
